# revision 15
# baseline (speedup 1.0000x reference)
"""Trainium2 Bass kernel for the DPRNN block (channel-norm -> unfold ->
4x bidirectional SRU -> conv-transpose -> residual).

Sharding: data-parallel over the B*T=512 sequences; 64 sequences per core.
All weights replicated. Each core runs the full pipeline on its shard.

v2: fp8-e4m3 DoubleRow matmuls (2 contraction rows/partition/cycle) for the
SRU layers, with a residual-W correction matmul on the z/hp outputs to kill
weight-quantization noise (f/r gate noise is damped by the sigmoid).  Conv
stays bf16.  Elementwise work is spread: Act = sigmoids + z evac, Pool =
hp evac + fp8 casts, DVE = gate algebra + scan + highway.  Weights are
scaled x16 into fp8 and the 1/16 folded into the evacuation scale slots.
"""
import os
import numpy as np
import ml_dtypes

import bass_rust
import concourse.bass as bass
import concourse.mybir as mybir
import concourse.tile as tile
from concourse import bacc
from concourse import bass_utils

F32 = mybir.dt.float32
F32R = mybir.dt.float32r
BF16 = mybir.dt.bfloat16
FP8 = mybir.dt.float8e4

B, C, T, F_ = 4, 64, 128, 128
H, K = 128, 8
L = F_ - K + 1            # 121
EPS = 1e-8
NCORES = 8
NLOC = (B * T) // NCORES  # 64 sequences per core
NF = NLOC * 128           # 8192
XCOLS = NF + 8            # xn2 / hb3 tiles carry 8 extra cols for shifts
UNIT = 2048               # cols per SRU compute unit
NUNIT = NF // UNIT        # 4
SPAN = 1024               # psum evacuation span
NCH = 8                   # norm apply chunks (1024 cols each)
WS = 16.0                 # weight scale into fp8

_CACHE = {}


def _pair_ap(t_ap, start, stride):
    """[128, 2, 512] AP with pair elements `stride` cols apart (overlapping)."""
    v = t_ap[:, start:start + stride + 512]
    a = v.copy()
    part = v.ap.to_list()[0]
    a.ap = bass_rust.VecI64Pair([part, [stride, 2], [1, 512]])
    return a


def _build():
    nc = bacc.Bacc("TRN2", target_bir_lowering=False, debug=False)
    AF = mybir.ActivationFunctionType
    OP = mybir.AluOpType
    DR = mybir.MatmulPerfMode.DoubleRow

    # ---------------- DRAM tensors ----------------
    u_d = nc.dram_tensor("u", [C, NLOC * F_], F32, kind="ExternalInput").ap()
    un_d = nc.dram_tensor("un", [NLOC, F_, C], F32, kind="ExternalInput").ap()
    w08_d = nc.dram_tensor("w08", [2, 2, 2, 128, 512], FP8, kind="ExternalInput").ap()
    w0r8_d = nc.dram_tensor("w0r8", [2, 2, 2, 128, 256], FP8, kind="ExternalInput").ap()
    wi8_d = nc.dram_tensor("wi8", [3, 2, 2, 128, 512], FP8, kind="ExternalInput").ap()
    wir8_d = nc.dram_tensor("wir8", [3, 2, 2, 128, 256], FP8, kind="ExternalInput").ap()
    cw_d = nc.dram_tensor("cwp", [2, 8, 128, 64], BF16, kind="ExternalInput").ap()
    bf_d = nc.dram_tensor("bfp", [4, 2, 128], F32, kind="ExternalInput").ap()
    br_d = nc.dram_tensor("brp", [4, 2, 128], F32, kind="ExternalInput").ap()
    gm2_d = nc.dram_tensor("gm2", [2, 128], F32, kind="ExternalInput").ap()
    btc_d = nc.dram_tensor("btc", [128], F32, kind="ExternalInput").ap()
    cb_d = nc.dram_tensor("cb", [C], F32, kind="ExternalInput").ap()
    out_d = nc.dram_tensor("o", [C, NF], F32, kind="ExternalOutput").ap()

    with tile.TileContext(nc) as tc:
        with tc.tile_pool(name="const", bufs=1) as cp:
            # ---- weights / biases resident in SBUF ----
            bfp_t = cp.tile([128, 8], F32)
            nc.sync.dma_start(bfp_t[:].rearrange("p (i d) -> p i d", i=4), bf_d.rearrange("i d p -> p i d"))
            brp_t = cp.tile([128, 8], F32)
            nc.sync.dma_start(brp_t[:].rearrange("p (i d) -> p i d", i=4), br_d.rearrange("i d p -> p i d"))
            gm2_t = cp.tile([2, 128], F32)
            nc.sync.dma_start(gm2_t[:], gm2_d)
            btc_t = cp.tile([128, 1], F32)
            nc.sync.dma_start(btc_t[:], btc_d.rearrange("(p a) -> p a", a=1))
            cb_t = cp.tile([C, 1], F32)
            nc.sync.dma_start(cb_t[:], cb_d.rearrange("(c a) -> c a", a=1))

            w08_t = cp.tile([128, 2 * 2 * 2 * 512], FP8)
            w08_v = w08_t[:].rearrange("p (d g pr m) -> p d g pr m", d=2, g=2, pr=2)
            nc.scalar.dma_start(w08_v, w08_d.rearrange("d g pr p m -> p d g pr m"))
            w0r8_t = cp.tile([128, 2 * 2 * 2 * 256], FP8)
            w0r8_v = w0r8_t[:].rearrange("p (d g pr m) -> p d g pr m", d=2, g=2, pr=2)
            nc.scalar.dma_start(w0r8_v, w0r8_d.rearrange("d g pr p m -> p d g pr m"))
            wi8_t = cp.tile([128, 3 * 2 * 2 * 512], FP8)
            wi8_v = wi8_t[:].rearrange("p (i d ct m) -> p i d ct m", i=3, d=2, ct=2)
            nc.scalar.dma_start(wi8_v, wi8_d.rearrange("i d ct p m -> p i d ct m"))
            wir8_t = cp.tile([128, 3 * 2 * 2 * 256], FP8)
            wir8_v = wir8_t[:].rearrange("p (i d ct m) -> p i d ct m", i=3, d=2, ct=2)
            nc.scalar.dma_start(wir8_v, wir8_d.rearrange("i d ct p m -> p i d ct m"))
            cw_t = cp.tile([128, 2 * 8 * 64], BF16)
            cw_v = cw_t[:].rearrange("p (ct k m) -> p ct k m", ct=2, k=8)
            nc.scalar.dma_start(cw_v, cw_d.rearrange("ct k p m -> p ct k m"))

            # ---- long-lived activations ----
            xnf_t = cp.tile([128, NF], F32)       # rows 0:64 xn, 64:128 xn<<1
            xn2_t = cp.tile([128, XCOLS], FP8)    # L0 rhs (plain ; shifted)
            h8_t = [cp.tile([128, 2 * NF], FP8, name=f"h8{i}") for i in range(2)]
            hb3_t = [cp.tile([128, XCOLS], BF16, name=f"hb3{i}") for i in range(2)]

            nc.gpsimd.memset(xn2_t[:, NF:XCOLS], 0.0)

            # ================= channel norm =================
            with (
                tc.tile_pool(name="stats", bufs=1) as stp,
                tc.tile_pool(name="a1p", bufs=2) as a1p,
                tc.tile_pool(name="u2p", bufs=2) as u2p,
                tc.tile_pool(name="npp", bufs=2, space="PSUM") as npp,
            ):
                un_t = stp.tile([128, 4096], F32)
                nc.sync.dma_start(
                    un_t[:].rearrange("p (fl c) -> p fl c", c=C),
                    un_d.rearrange("n (fh fl) c -> (n fh) fl c", fh=2),
                )
                sq_t = stp.tile([128, 4096], F32)
                nc.scalar.activation(sq_t[:], un_t[:], AF.Square)
                mu_t = stp.tile([128, 64], F32)
                s2_t = stp.tile([128, 64], F32)
                un_v = un_t[:].rearrange("p (fl c) -> p fl c", c=C)
                sq_v = sq_t[:].rearrange("p (fl c) -> p fl c", c=C)
                nc.vector.tensor_reduce(mu_t[:], un_v, axis=mybir.AxisListType.X, op=OP.add)
                nc.vector.tensor_reduce(s2_t[:], sq_v, axis=mybir.AxisListType.X, op=OP.add)
                nc.vector.tensor_scalar_mul(mu_t[:], mu_t[:], 1.0 / C)
                tmp_t = stp.tile([128, 64], F32)
                nc.vector.tensor_mul(tmp_t[:], mu_t[:], mu_t[:])
                nc.vector.scalar_tensor_tensor(
                    s2_t[:], s2_t[:], 1.0 / C, tmp_t[:], op0=OP.mult, op1=OP.subtract
                )  # var = s2/C - mu^2
                A_t = stp.tile([128, 64], F32)
                B_t = stp.tile([128, 64], F32)
                eps_t = stp.tile([128, 1], F32)
                nc.vector.memset(eps_t[:], EPS)
                nc.scalar.activation(tmp_t[:], s2_t[:], AF.Sqrt, bias=eps_t[:, 0:1])
                nc.vector.reciprocal(A_t[:], tmp_t[:])
                nc.vector.scalar_tensor_tensor(
                    B_t[:], mu_t[:], -1.0, A_t[:], op0=OP.mult, op1=OP.mult
                )
                PCH = SPAN // 64  # A_t partitions per 1024-col chunk

                for ch in range(NCH):
                    sl = slice(ch * SPAN, (ch + 1) * SPAN)
                    n1 = SPAN if ch < NCH - 1 else SPAN - 1
                    pr = slice(ch * PCH, (ch + 1) * PCH)
                    a1 = a1p.tile([2, SPAN], F32, tag="a1")
                    b1 = a1p.tile([2, SPAN], F32, tag="b1")
                    if ch == NCH - 1:
                        nc.vector.memset(a1[:, SPAN - 1:SPAN], 0.0)
                        nc.vector.memset(b1[:, SPAN - 1:SPAN], 0.0)
                    nc.sync.dma_start(a1[0:1, :], A_t[pr, :])
                    nc.sync.dma_start(b1[0:1, :], B_t[pr, :])
                    # shifted row: self-copy row0<<1 plus the boundary element
                    nc.sync.dma_start(a1[1:2, 0:SPAN - 1], a1[0:1, 1:SPAN])
                    nc.sync.dma_start(b1[1:2, 0:SPAN - 1], b1[0:1, 1:SPAN])
                    if ch < NCH - 1:
                        nc.sync.dma_start(a1[1:2, SPAN - 1:SPAN],
                                          A_t[(ch + 1) * PCH:(ch + 1) * PCH + 1, 0:1])
                        nc.sync.dma_start(b1[1:2, SPAN - 1:SPAN],
                                          B_t[(ch + 1) * PCH:(ch + 1) * PCH + 1, 0:1])
                    u2 = u2p.tile([128, SPAN], F32, tag="u2")
                    nc.scalar.dma_start(u2[0:64, :], u_d[:, sl])
                    nc.scalar.dma_start(u2[64:128, 0:n1], u_d[:, ch * SPAN + 1:ch * SPAN + 1 + n1])
                    if ch == NCH - 1:
                        nc.vector.memset(u2[64:128, n1:SPAN], 0.0)
                    ag = npp.tile([128, SPAN], F32, tag="ag")
                    bg = npp.tile([128, SPAN], F32, tag="bg")
                    for h2 in range(2):
                        psl = slice(h2 * 512, (h2 + 1) * 512)
                        nc.tensor.matmul(ag[:, psl], gm2_t[:],
                                         a1[:, psl], start=True, stop=True)
                        nc.tensor.matmul(bg[:, psl], gm2_t[:],
                                         b1[:, psl], start=True, stop=True)
                    nc.vector.tensor_mul(xnf_t[:, sl], u2[:], ag[:])
                    nc.vector.scalar_tensor_tensor(
                        xnf_t[:, sl], xnf_t[:, sl], btc_t[:, 0:1], bg[:],
                        op0=OP.add, op1=OP.add,
                    )
                    nc.scalar.copy(xn2_t[:, sl], xnf_t[:, sl])

            # ================= SRU layers =================
            sig = AF.Sigmoid
            KINDS = (0, 1, 2, 3)  # z, f, r, hp
            with (
                tc.tile_pool(name="gates", bufs=2) as gp,
                tc.tile_pool(name="hscr", bufs=2) as sp,
                tc.tile_pool(name="lps", bufs=1, space="PSUM") as pp,
            ):
                for li in range(4):
                    for q in range(NUNIT):
                        for d in range(2):
                            base = q * UNIT
                            bcol = bfp_t[:, 2 * li + d:2 * li + d + 1]
                            rcol = brp_t[:, 2 * li + d:2 * li + d + 1]
                            z_t = gp.tile([128, UNIT], BF16, tag="z")
                            g_t = gp.tile([128, UNIT], BF16, tag="g")
                            r_t = gp.tile([128, UNIT], BF16, tag="r")
                            w_t = gp.tile([128, UNIT], BF16, tag="w")
                            for s in range(2):
                                sb = base + s * SPAN
                                ps = [pp.tile([128, SPAN], F32, name=f"ps{o}", tag=f"ps{o}")
                                      for o in KINDS]
                                for o in KINDS:
                                    resid = o in (0, 3)
                                    ro = 0 if o == 0 else 1
                                    for c2 in range(2):
                                        col = sb + c2 * 512
                                        osl = ps[o][:, c2 * 512:(c2 + 1) * 512]
                                        if li == 0:
                                            nmm = 4 if resid else 2
                                            mm = 0
                                            for g2 in range(2):
                                                rhs = _pair_ap(xn2_t, col + 4 * g2, 2)
                                                nc.tensor.matmul(
                                                    osl, w08_v[:, d, g2, :, o * 128:(o + 1) * 128],
                                                    rhs, start=(mm == 0), stop=(mm == nmm - 1),
                                                    perf_mode=DR)
                                                mm += 1
                                            if resid:
                                                for g2 in range(2):
                                                    rhs = _pair_ap(xn2_t, col + 4 * g2, 2)
                                                    nc.tensor.matmul(
                                                        osl, w0r8_v[:, d, g2, :, ro * 128:(ro + 1) * 128],
                                                        rhs, start=False, stop=(mm == nmm - 1),
                                                        perf_mode=DR)
                                                    mm += 1
                                        else:
                                            h8v = h8_t[(li - 1) % 2][:].rearrange(
                                                "p (ct x) -> p ct x", ct=2)
                                            rhs = h8v[:, :, col:col + 512]
                                            nmm = 2 if resid else 1
                                            nc.tensor.matmul(
                                                osl, wi8_v[:, li - 1, d, :, o * 128:(o + 1) * 128],
                                                rhs, start=True, stop=(nmm == 1), perf_mode=DR)
                                            if resid:
                                                nc.tensor.matmul(
                                                    osl, wir8_v[:, li - 1, d, :, ro * 128:(ro + 1) * 128],
                                                    rhs, start=False, stop=True, perf_mode=DR)
                                # evacuate span (z first so PE can reuse its bank)
                                ssl = slice(s * SPAN, (s + 1) * SPAN)
                                srcs = [p_[:] for p_ in ps]
                                if d == 1:
                                    srcs = [p_[:].rearrange("p (n l) -> p n l", l=128)[:, :, ::-1]
                                            for p_ in ps]
                                nc.scalar.activation(z_t[:, ssl], srcs[0], AF.Copy, scale=1.0 / WS)
                                nc.scalar.activation(g_t[:, ssl], srcs[1], sig,
                                                     bias=bcol, scale=-1.0 / WS)
                                nc.scalar.activation(r_t[:, ssl], srcs[2], sig,
                                                     bias=rcol, scale=1.0 / WS)
                                nc.scalar.activation(w_t[:, ssl], srcs[3], AF.Copy,
                                                     scale=1.0 / WS)
                            # b = g*z (read g before 1-g overwrites it)
                            nc.vector.tensor_mul(z_t[:], g_t[:], z_t[:])
                            nc.vector.tensor_scalar(g_t[:], g_t[:], -1.0, 1.0,
                                                    op0=OP.mult, op1=OP.add)  # f = 1-g
                            g_v = g_t[:].rearrange("p (n l) -> p n l", l=128)
                            z_v = z_t[:].rearrange("p (n l) -> p n l", l=128)
                            pads = slice(121, 128) if d == 0 else slice(0, 7)
                            nc.gpsimd.memset(g_v[:, :, pads], 0.0)
                            nc.gpsimd.memset(z_v[:, :, pads], 0.0)
                            # c = f*c + b
                            nc.vector.tensor_tensor_scan(
                                z_t[:], g_t[:], z_t[:], 0.0, op0=OP.mult, op1=OP.add)
                            # highway: out = r*(c-hp) + hp
                            nc.vector.tensor_sub(g_t[:], z_t[:], w_t[:])
                            nc.vector.tensor_mul(r_t[:], r_t[:], g_t[:])
                            r_v = r_t[:].rearrange("p (n l) -> p n l", l=128)
                            w_v = w_t[:].rearrange("p (n l) -> p n l", l=128)
                            if li < 3:
                                hs = sp.tile([128, UNIT], BF16, tag="hb")
                                dst = hs[:].rearrange("p (n l) -> p n l", l=128)
                                if d == 1:
                                    dst = dst[:, :, ::-1]
                                nc.vector.tensor_add(dst, r_v, w_v)
                                h8o = h8_t[li % 2][:].rearrange("p (ct x) -> p ct x", ct=2)
                                nc.gpsimd.tensor_scalar_mul(
                                    h8o[:, d, base:base + UNIT], hs[:], 1.0)
                            else:
                                dst = hb3_t[d][:, 8 + base:8 + base + UNIT].rearrange(
                                    "p (n l) -> p n l", l=128)
                                if d == 1:
                                    dst = dst[:, :, ::-1]
                                nc.vector.tensor_add(dst, r_v, w_v)

            # ================= transposed conv + residual =================
            for t4 in hb3_t:
                v = t4[:, 0:NF].rearrange("p (n l) -> p n l", l=128)
                nc.gpsimd.memset(t4[:, 0:8], 0.0)
                nc.gpsimd.memset(v[:, 1:33, 1:8], 0.0)
                nc.gpsimd.memset(v[:, 33:64, 1:8], 0.0)
                nc.gpsimd.memset(t4[:, NF + 1:XCOLS], 0.0)
            with (
                tc.tile_pool(name="cvp", bufs=4, space="PSUM") as cvp,
                tc.tile_pool(name="osp", bufs=2) as osp,
            ):
                for span in range(NF // SPAN):
                    c_ps = cvp.tile([C, SPAN], F32, tag="c")
                    for h2 in range(2):
                        osl = c_ps[:, h2 * 512:(h2 + 1) * 512]
                        cbase = span * SPAN + h2 * 512
                        mm = 0
                        for ct in range(2):
                            for k in range(8):
                                rhs = hb3_t[ct][:, 8 - k + cbase:8 - k + cbase + 512]
                                nc.tensor.matmul(
                                    osl, cw_v[:, ct, k, :], rhs,
                                    start=(mm == 0), stop=(mm == 15))
                                mm += 1
                    o_t = osp.tile([C, SPAN], F32, tag="o")
                    sl = slice(span * SPAN, (span + 1) * SPAN)
                    nc.vector.scalar_tensor_tensor(
                        o_t[:], c_ps[:], cb_t[:, 0:1], xnf_t[0:64, sl],
                        op0=OP.add, op1=OP.add,
                    )
                    nc.sync.dma_start(out_d[:, sl], o_t[:])

    nc.compile()
    return nc


def _prep_weights(W0, Ws, convW):
    f8 = ml_dtypes.float8_e4m3

    def q8(x):
        return x.astype(f8).astype(np.float32)

    # layer 0: chunks ct cover k-offsets (2ct, 2ct+1); partition rows 0:64
    # even-k (plain xn2 rows), 64:128 odd-k (shifted rows). DR pair g joins
    # chunks (2g, 2g+1).
    w0r = W0.reshape(C, K, 2, 4 * H)
    w0p = np.zeros((2, 4, 128, 512), np.float32)
    for d in range(2):
        for ct in range(4):
            w0p[d, ct, 0:64] = w0r[:, 2 * ct, d]
            w0p[d, ct, 64:128] = w0r[:, 2 * ct + 1, d]
    w0s = w0p * WS
    w08 = np.zeros((2, 2, 2, 128, 512), np.float32)
    w0r8 = np.zeros((2, 2, 2, 128, 256), np.float32)
    for d in range(2):
        for g in range(2):
            for pr in range(2):
                m = q8(w0s[d, 2 * g + pr])
                w08[d, g, pr] = m
                dw = q8(w0s[d, 2 * g + pr] - m)
                w0r8[d, g, pr, :, 0:128] = dw[:, 0:128]       # z resid
                w0r8[d, g, pr, :, 128:256] = dw[:, 384:512]   # hp resid
    # layers 1-3
    wi8 = np.zeros((3, 2, 2, 128, 512), np.float32)
    wir8 = np.zeros((3, 2, 2, 128, 256), np.float32)
    for i in range(3):
        for d in range(2):
            for ct in range(2):
                ws = Ws[i][ct * 128:(ct + 1) * 128, d] * WS
                m = q8(ws)
                wi8[i, d, ct] = m
                dw = q8(ws - m)
                wir8[i, d, ct, :, 0:128] = dw[:, 0:128]
                wir8[i, d, ct, :, 128:256] = dw[:, 384:512]
    cwp = np.zeros((2, 8, 128, C), np.float32)
    for ct in range(2):
        for k in range(8):
            cwp[ct, k] = convW[ct * 128:(ct + 1) * 128, :, k]
    bf16 = ml_dtypes.bfloat16
    return (w08.astype(f8), w0r8.astype(f8), wi8.astype(f8), wir8.astype(f8),
            cwp.astype(bf16))


def kernel(**inputs):
    inputs = {k: np.asarray(v) for k, v in inputs.items()}
    x = inputs["x"].astype(np.float32)
    xs = np.ascontiguousarray(
        x.transpose(0, 2, 1, 3).reshape(B * T, C, F_)
    )  # (512, C, F)

    w08, w0r8, wi8, wir8, cwp = _prep_weights(
        inputs["W0"].astype(np.float32),
        [inputs[f"W{i}"].astype(np.float32) for i in (1, 2, 3)],
        inputs["convW"].astype(np.float32),
    )
    bfp = -np.stack([inputs[f"bf{i}"] for i in range(4)]).astype(np.float32)
    brp = np.stack([inputs[f"br{i}"] for i in range(4)]).astype(np.float32)
    gm = inputs["gamma"].reshape(C).astype(np.float32)
    bt = inputs["beta"].reshape(C).astype(np.float32)
    cb = inputs["convb"].reshape(C).astype(np.float32)
    gm2 = np.zeros((2, 128), np.float32)
    gm2[0, 0:64] = gm
    gm2[1, 64:128] = gm
    btc = np.concatenate([bt, bt]).astype(np.float32)

    if "nc" not in _CACHE:
        _CACHE["nc"] = _build()
    nc = _CACHE["nc"]

    shared = {"w08": w08, "w0r8": w0r8, "wi8": wi8, "wir8": wir8, "cwp": cwp,
              "bfp": bfp, "brp": brp, "gm2": gm2, "btc": btc, "cb": cb}
    in_maps = []
    for core in range(NCORES):
        sh = xs[core * NLOC:(core + 1) * NLOC]  # (NLOC, C, F)
        u = np.ascontiguousarray(sh.transpose(1, 0, 2)).reshape(C, NLOC * F_)
        un = np.ascontiguousarray(sh.transpose(0, 2, 1))  # (NLOC, F, C)
        in_maps.append({"u": u, "un": un, **shared})

    trace = bool(os.environ.get("KBENCH_TRACE"))
    res = bass_utils.run_bass_kernel_spmd(
        nc, in_maps, list(range(NCORES)), trace=trace,
        tmpdir=os.environ.get("KBENCH_TMPDIR"),
    )
    _CACHE["last_result"] = res

    full = np.concatenate(
        [res.results[i]["o"].reshape(C, NLOC, F_) for i in range(NCORES)], axis=1
    )  # (C, 512, F)
    out = full.transpose(1, 0, 2).reshape(B, T, C, F_).transpose(0, 2, 1, 3)
    return np.ascontiguousarray(out.astype(np.float32))


# revision 16
# speedup vs baseline: 2.2412x; 2.2412x over previous
"""Trainium2 Bass kernel for the DPRNN block (channel-norm -> unfold ->
4x bidirectional SRU -> conv-transpose -> residual).

Sharding: data-parallel over the B*T=512 sequences; 64 sequences per core.
All weights replicated. Each core runs the full pipeline on its shard.

v2: fp8-e4m3 DoubleRow matmuls (2 contraction rows/partition/cycle) for the
SRU layers, with a residual-W correction matmul on the z/hp outputs to kill
weight-quantization noise (f/r gate noise is damped by the sigmoid).  Conv
stays bf16.  Elementwise work is spread: Act = sigmoids + z evac, Pool =
hp evac + fp8 casts, DVE = gate algebra + scan + highway.  Weights are
scaled x16 into fp8 and the 1/16 folded into the evacuation scale slots.
"""
import os
import numpy as np
import ml_dtypes

import bass_rust
import concourse.bass as bass
import concourse.mybir as mybir
import concourse.tile as tile
from concourse import bacc
from concourse import bass_utils

F32 = mybir.dt.float32
F32R = mybir.dt.float32r
BF16 = mybir.dt.bfloat16
FP8 = mybir.dt.float8e4

B, C, T, F_ = 4, 64, 128, 128
H, K = 128, 8
L = F_ - K + 1            # 121
EPS = 1e-8
NCORES = 8
NLOC = (B * T) // NCORES  # 64 sequences per core
NF = NLOC * 128           # 8192
XCOLS = NF + 8            # xn2 / hb3 tiles carry 8 extra cols for shifts
UNIT = 2048               # cols per SRU compute unit
NUNIT = NF // UNIT        # 4
SPAN = 1024               # psum evacuation span
NCH = 8                   # norm apply chunks (1024 cols each)
WS = 16.0                 # weight scale into fp8

_CACHE = {}


def _pair_ap(t_ap, start, stride):
    """[128, 2, 512] AP with pair elements `stride` cols apart (overlapping)."""
    v = t_ap[:, start:start + stride + 512]
    a = v.copy()
    part = v.ap.to_list()[0]
    a.ap = bass_rust.VecI64Pair([part, [stride, 2], [1, 512]])
    return a


def _build():
    nc = bacc.Bacc("TRN2", target_bir_lowering=False, debug=False)
    AF = mybir.ActivationFunctionType
    OP = mybir.AluOpType
    DR = mybir.MatmulPerfMode.DoubleRow

    # ---------------- DRAM tensors ----------------
    u_d = nc.dram_tensor("u", [C, NLOC * F_], F32, kind="ExternalInput").ap()
    un_d = nc.dram_tensor("un", [NLOC, F_, C], F32, kind="ExternalInput").ap()
    w08_d = nc.dram_tensor("w08", [2, 2, 2, 128, 512], FP8, kind="ExternalInput").ap()
    w0r8_d = nc.dram_tensor("w0r8", [2, 2, 2, 128, 256], FP8, kind="ExternalInput").ap()
    wi8_d = nc.dram_tensor("wi8", [3, 2, 2, 128, 512], FP8, kind="ExternalInput").ap()
    wir8_d = nc.dram_tensor("wir8", [3, 2, 2, 128, 256], FP8, kind="ExternalInput").ap()
    cw_d = nc.dram_tensor("cwp", [2, 8, 128, 64], BF16, kind="ExternalInput").ap()
    bf_d = nc.dram_tensor("bfp", [4, 2, 128], F32, kind="ExternalInput").ap()
    br_d = nc.dram_tensor("brp", [4, 2, 128], F32, kind="ExternalInput").ap()
    gm2_d = nc.dram_tensor("gm2", [2, 128], F32, kind="ExternalInput").ap()
    btc_d = nc.dram_tensor("btc", [128], F32, kind="ExternalInput").ap()
    cb_d = nc.dram_tensor("cb", [C], F32, kind="ExternalInput").ap()
    out_d = nc.dram_tensor("o", [C, NF], F32, kind="ExternalOutput").ap()

    with tile.TileContext(nc) as tc:
        with tc.tile_pool(name="const", bufs=1) as cp:
            # ---- weights / biases resident in SBUF ----
            bfp_t = cp.tile([128, 8], F32)
            nc.sync.dma_start(bfp_t[:].rearrange("p (i d) -> p i d", i=4), bf_d.rearrange("i d p -> p i d"))
            brp_t = cp.tile([128, 8], F32)
            nc.sync.dma_start(brp_t[:].rearrange("p (i d) -> p i d", i=4), br_d.rearrange("i d p -> p i d"))
            gm2_t = cp.tile([2, 128], F32)
            nc.sync.dma_start(gm2_t[:], gm2_d)
            btc_t = cp.tile([128, 1], F32)
            nc.sync.dma_start(btc_t[:], btc_d.rearrange("(p a) -> p a", a=1))
            cb_t = cp.tile([C, 1], F32)
            nc.sync.dma_start(cb_t[:], cb_d.rearrange("(c a) -> c a", a=1))

            w08_t = cp.tile([128, 2 * 2 * 2 * 512], FP8)
            w08_v = w08_t[:].rearrange("p (d g pr m) -> p d g pr m", d=2, g=2, pr=2)
            nc.scalar.dma_start(w08_v, w08_d.rearrange("d g pr p m -> p d g pr m"))
            w0r8_t = cp.tile([128, 2 * 2 * 2 * 256], FP8)
            w0r8_v = w0r8_t[:].rearrange("p (d g pr m) -> p d g pr m", d=2, g=2, pr=2)
            nc.scalar.dma_start(w0r8_v, w0r8_d.rearrange("d g pr p m -> p d g pr m"))
            wi8_t = cp.tile([128, 3 * 2 * 2 * 512], FP8)
            wi8_v = wi8_t[:].rearrange("p (i d ct m) -> p i d ct m", i=3, d=2, ct=2)
            nc.scalar.dma_start(wi8_v, wi8_d.rearrange("i d ct p m -> p i d ct m"))
            wir8_t = cp.tile([128, 3 * 2 * 2 * 256], FP8)
            wir8_v = wir8_t[:].rearrange("p (i d ct m) -> p i d ct m", i=3, d=2, ct=2)
            nc.scalar.dma_start(wir8_v, wir8_d.rearrange("i d ct p m -> p i d ct m"))
            cw_t = cp.tile([128, 2 * 8 * 64], BF16)
            cw_v = cw_t[:].rearrange("p (ct k m) -> p ct k m", ct=2, k=8)
            nc.scalar.dma_start(cw_v, cw_d.rearrange("ct k p m -> p ct k m"))

            # ---- long-lived activations ----
            xnf_t = cp.tile([128, NF], F32)       # rows 0:64 xn, 64:128 xn<<1
            xn2_t = cp.tile([128, XCOLS], FP8)    # L0 rhs (plain ; shifted)
            h8_t = [cp.tile([128, 2 * NF], FP8, name=f"h8{i}") for i in range(2)]
            hb3_t = [cp.tile([128, XCOLS], BF16, name=f"hb3{i}") for i in range(2)]

            nc.gpsimd.memset(xn2_t[:, NF:XCOLS], 0.0)

            # ================= channel norm =================
            with (
                tc.tile_pool(name="stats", bufs=1) as stp,
                tc.tile_pool(name="a1p", bufs=2) as a1p,
                tc.tile_pool(name="u2p", bufs=2) as u2p,
                tc.tile_pool(name="npp", bufs=2, space="PSUM") as npp,
            ):
                un_t = stp.tile([128, 4096], F32)
                nc.sync.dma_start(
                    un_t[:].rearrange("p (fl c) -> p fl c", c=C),
                    un_d.rearrange("n (fh fl) c -> (n fh) fl c", fh=2),
                )
                sq_t = stp.tile([128, 4096], F32)
                nc.scalar.activation(sq_t[:], un_t[:], AF.Square)
                mu_t = stp.tile([128, 64], F32)
                s2_t = stp.tile([128, 64], F32)
                un_v = un_t[:].rearrange("p (fl c) -> p fl c", c=C)
                sq_v = sq_t[:].rearrange("p (fl c) -> p fl c", c=C)
                nc.vector.tensor_reduce(mu_t[:], un_v, axis=mybir.AxisListType.X, op=OP.add)
                nc.vector.tensor_reduce(s2_t[:], sq_v, axis=mybir.AxisListType.X, op=OP.add)
                nc.vector.tensor_scalar_mul(mu_t[:], mu_t[:], 1.0 / C)
                tmp_t = stp.tile([128, 64], F32)
                nc.vector.tensor_mul(tmp_t[:], mu_t[:], mu_t[:])
                nc.vector.scalar_tensor_tensor(
                    s2_t[:], s2_t[:], 1.0 / C, tmp_t[:], op0=OP.mult, op1=OP.subtract
                )  # var = s2/C - mu^2
                A_t = stp.tile([128, 64], F32)
                B_t = stp.tile([128, 64], F32)
                eps_t = stp.tile([128, 1], F32)
                nc.vector.memset(eps_t[:], EPS)
                nc.scalar.activation(tmp_t[:], s2_t[:], AF.Sqrt, bias=eps_t[:, 0:1])
                nc.vector.reciprocal(A_t[:], tmp_t[:])
                nc.vector.scalar_tensor_tensor(
                    B_t[:], mu_t[:], -1.0, A_t[:], op0=OP.mult, op1=OP.mult
                )
                PCH = SPAN // 64  # A_t partitions per 1024-col chunk

                for ch in range(NCH):
                    sl = slice(ch * SPAN, (ch + 1) * SPAN)
                    n1 = SPAN if ch < NCH - 1 else SPAN - 1
                    pr = slice(ch * PCH, (ch + 1) * PCH)
                    a1 = a1p.tile([2, SPAN], F32, tag="a1")
                    b1 = a1p.tile([2, SPAN], F32, tag="b1")
                    if ch == NCH - 1:
                        nc.vector.memset(a1[:, SPAN - 1:SPAN], 0.0)
                        nc.vector.memset(b1[:, SPAN - 1:SPAN], 0.0)
                    nc.sync.dma_start(a1[0:1, :], A_t[pr, :])
                    nc.sync.dma_start(b1[0:1, :], B_t[pr, :])
                    # shifted row: self-copy row0<<1 plus the boundary element
                    nc.sync.dma_start(a1[1:2, 0:SPAN - 1], a1[0:1, 1:SPAN])
                    nc.sync.dma_start(b1[1:2, 0:SPAN - 1], b1[0:1, 1:SPAN])
                    if ch < NCH - 1:
                        nc.sync.dma_start(a1[1:2, SPAN - 1:SPAN],
                                          A_t[(ch + 1) * PCH:(ch + 1) * PCH + 1, 0:1])
                        nc.sync.dma_start(b1[1:2, SPAN - 1:SPAN],
                                          B_t[(ch + 1) * PCH:(ch + 1) * PCH + 1, 0:1])
                    u2 = u2p.tile([128, SPAN], F32, tag="u2")
                    nc.scalar.dma_start(u2[0:64, :], u_d[:, sl])
                    nc.scalar.dma_start(u2[64:128, 0:n1], u_d[:, ch * SPAN + 1:ch * SPAN + 1 + n1])
                    if ch == NCH - 1:
                        nc.vector.memset(u2[64:128, n1:SPAN], 0.0)
                    ag = npp.tile([128, SPAN], F32, tag="ag")
                    bg = npp.tile([128, SPAN], F32, tag="bg")
                    for h2 in range(2):
                        psl = slice(h2 * 512, (h2 + 1) * 512)
                        nc.tensor.matmul(ag[:, psl], gm2_t[:],
                                         a1[:, psl], start=True, stop=True)
                        nc.tensor.matmul(bg[:, psl], gm2_t[:],
                                         b1[:, psl], start=True, stop=True)
                    nc.vector.tensor_mul(xnf_t[:, sl], u2[:], ag[:])
                    nc.vector.scalar_tensor_tensor(
                        xnf_t[:, sl], xnf_t[:, sl], btc_t[:, 0:1], bg[:],
                        op0=OP.add, op1=OP.add,
                    )
                    nc.scalar.copy(xn2_t[:, sl], xnf_t[:, sl])

            # ================= SRU layers =================
            sig = AF.Sigmoid
            KINDS = (0, 1, 2, 3)  # z, f, r, hp
            with (
                tc.tile_pool(name="gates", bufs=2) as gp,
                tc.tile_pool(name="hscr", bufs=2) as sp,
                tc.tile_pool(name="lps", bufs=1, space="PSUM") as pp,
            ):
                for li in range(4):
                    for q in range(NUNIT):
                        for d in range(2):
                            base = q * UNIT
                            bcol = bfp_t[:, 2 * li + d:2 * li + d + 1]
                            rcol = brp_t[:, 2 * li + d:2 * li + d + 1]
                            z_t = gp.tile([128, UNIT], BF16, tag="z")
                            g_t = gp.tile([128, UNIT], BF16, tag="g")
                            r_t = gp.tile([128, UNIT], BF16, tag="r")
                            w_t = gp.tile([128, UNIT], BF16, tag="w")
                            for s in range(2):
                                sb = base + s * SPAN
                                ps = [pp.tile([128, SPAN], F32, name=f"ps{o}", tag=f"ps{o}")
                                      for o in KINDS]
                                for o in KINDS:
                                    resid = o in (0, 3)
                                    ro = 0 if o == 0 else 1
                                    for c2 in range(2):
                                        col = sb + c2 * 512
                                        osl = ps[o][:, c2 * 512:(c2 + 1) * 512]
                                        if li == 0:
                                            nmm = 4 if resid else 2
                                            mm = 0
                                            for g2 in range(2):
                                                rhs = _pair_ap(xn2_t, col + 4 * g2, 2)
                                                nc.tensor.matmul(
                                                    osl, w08_v[:, d, g2, :, o * 128:(o + 1) * 128],
                                                    rhs, start=(mm == 0), stop=(mm == nmm - 1),
                                                    perf_mode=DR)
                                                mm += 1
                                            if resid:
                                                for g2 in range(2):
                                                    rhs = _pair_ap(xn2_t, col + 4 * g2, 2)
                                                    nc.tensor.matmul(
                                                        osl, w0r8_v[:, d, g2, :, ro * 128:(ro + 1) * 128],
                                                        rhs, start=False, stop=(mm == nmm - 1),
                                                        perf_mode=DR)
                                                    mm += 1
                                        else:
                                            h8v = h8_t[(li - 1) % 2][:].rearrange(
                                                "p (ct x) -> p ct x", ct=2)
                                            rhs = h8v[:, :, col:col + 512]
                                            nmm = 2 if resid else 1
                                            nc.tensor.matmul(
                                                osl, wi8_v[:, li - 1, d, :, o * 128:(o + 1) * 128],
                                                rhs, start=True, stop=(nmm == 1), perf_mode=DR)
                                            if resid:
                                                nc.tensor.matmul(
                                                    osl, wir8_v[:, li - 1, d, :, ro * 128:(ro + 1) * 128],
                                                    rhs, start=False, stop=True, perf_mode=DR)
                                # evacuate span (z first so PE can reuse its bank)
                                ssl = slice(s * SPAN, (s + 1) * SPAN)
                                srcs = [p_[:] for p_ in ps]
                                if d == 1:
                                    srcs = [p_[:].rearrange("p (n l) -> p n l", l=128)[:, :, ::-1]
                                            for p_ in ps]
                                nc.scalar.activation(z_t[:, ssl], srcs[0], AF.Copy, scale=1.0 / WS)
                                nc.scalar.activation(g_t[:, ssl], srcs[1], sig,
                                                     bias=bcol, scale=-1.0 / WS)
                                nc.scalar.activation(r_t[:, ssl], srcs[2], sig,
                                                     bias=rcol, scale=1.0 / WS)
                                nc.scalar.activation(w_t[:, ssl], srcs[3], AF.Copy,
                                                     scale=1.0 / WS)
                            # b = g*z (read g before 1-g overwrites it)
                            nc.vector.tensor_mul(z_t[:], g_t[:], z_t[:])
                            nc.vector.tensor_scalar(g_t[:], g_t[:], -1.0, 1.0,
                                                    op0=OP.mult, op1=OP.add)  # f = 1-g
                            g_v = g_t[:].rearrange("p (n l) -> p n l", l=128)
                            z_v = z_t[:].rearrange("p (n l) -> p n l", l=128)
                            pads = slice(121, 128) if d == 0 else slice(0, 7)
                            nc.gpsimd.memset(g_v[:, :, pads], 0.0)
                            nc.gpsimd.memset(z_v[:, :, pads], 0.0)
                            # c = f*c + b
                            nc.vector.tensor_tensor_scan(
                                z_t[:], g_t[:], z_t[:], 0.0, op0=OP.mult, op1=OP.add)
                            # highway: out = r*(c-hp) + hp
                            nc.vector.tensor_sub(g_t[:], z_t[:], w_t[:])
                            nc.vector.tensor_mul(r_t[:], r_t[:], g_t[:])
                            r_v = r_t[:].rearrange("p (n l) -> p n l", l=128)
                            w_v = w_t[:].rearrange("p (n l) -> p n l", l=128)
                            if li < 3:
                                hs = sp.tile([128, UNIT], BF16, tag="hb")
                                dst = hs[:].rearrange("p (n l) -> p n l", l=128)
                                if d == 1:
                                    dst = dst[:, :, ::-1]
                                nc.vector.tensor_add(dst, r_v, w_v)
                                h8o = h8_t[li % 2][:].rearrange("p (ct x) -> p ct x", ct=2)
                                nc.scalar.copy(h8o[:, d, base:base + UNIT], hs[:])
                            else:
                                dst = hb3_t[d][:, 8 + base:8 + base + UNIT].rearrange(
                                    "p (n l) -> p n l", l=128)
                                if d == 1:
                                    dst = dst[:, :, ::-1]
                                nc.vector.tensor_add(dst, r_v, w_v)

            # ================= transposed conv + residual =================
            for t4 in hb3_t:
                v = t4[:, 0:NF].rearrange("p (n l) -> p n l", l=128)
                nc.gpsimd.memset(t4[:, 0:8], 0.0)
                nc.gpsimd.memset(v[:, 1:33, 1:8], 0.0)
                nc.gpsimd.memset(v[:, 33:64, 1:8], 0.0)
                nc.gpsimd.memset(t4[:, NF + 1:XCOLS], 0.0)
            with (
                tc.tile_pool(name="cvp", bufs=4, space="PSUM") as cvp,
                tc.tile_pool(name="osp", bufs=2) as osp,
            ):
                for span in range(NF // SPAN):
                    c_ps = cvp.tile([C, SPAN], F32, tag="c")
                    for h2 in range(2):
                        osl = c_ps[:, h2 * 512:(h2 + 1) * 512]
                        cbase = span * SPAN + h2 * 512
                        mm = 0
                        for ct in range(2):
                            for k in range(8):
                                rhs = hb3_t[ct][:, 8 - k + cbase:8 - k + cbase + 512]
                                nc.tensor.matmul(
                                    osl, cw_v[:, ct, k, :], rhs,
                                    start=(mm == 0), stop=(mm == 15))
                                mm += 1
                    o_t = osp.tile([C, SPAN], F32, tag="o")
                    sl = slice(span * SPAN, (span + 1) * SPAN)
                    nc.vector.scalar_tensor_tensor(
                        o_t[:], c_ps[:], cb_t[:, 0:1], xnf_t[0:64, sl],
                        op0=OP.add, op1=OP.add,
                    )
                    nc.sync.dma_start(out_d[:, sl], o_t[:])

    nc.compile()
    return nc


def _prep_weights(W0, Ws, convW):
    f8 = ml_dtypes.float8_e4m3

    def q8(x):
        return x.astype(f8).astype(np.float32)

    # layer 0: chunks ct cover k-offsets (2ct, 2ct+1); partition rows 0:64
    # even-k (plain xn2 rows), 64:128 odd-k (shifted rows). DR pair g joins
    # chunks (2g, 2g+1).
    w0r = W0.reshape(C, K, 2, 4 * H)
    w0p = np.zeros((2, 4, 128, 512), np.float32)
    for d in range(2):
        for ct in range(4):
            w0p[d, ct, 0:64] = w0r[:, 2 * ct, d]
            w0p[d, ct, 64:128] = w0r[:, 2 * ct + 1, d]
    w0s = w0p * WS
    w08 = np.zeros((2, 2, 2, 128, 512), np.float32)
    w0r8 = np.zeros((2, 2, 2, 128, 256), np.float32)
    for d in range(2):
        for g in range(2):
            for pr in range(2):
                m = q8(w0s[d, 2 * g + pr])
                w08[d, g, pr] = m
                dw = q8(w0s[d, 2 * g + pr] - m)
                w0r8[d, g, pr, :, 0:128] = dw[:, 0:128]       # z resid
                w0r8[d, g, pr, :, 128:256] = dw[:, 384:512]   # hp resid
    # layers 1-3
    wi8 = np.zeros((3, 2, 2, 128, 512), np.float32)
    wir8 = np.zeros((3, 2, 2, 128, 256), np.float32)
    for i in range(3):
        for d in range(2):
            for ct in range(2):
                ws = Ws[i][ct * 128:(ct + 1) * 128, d] * WS
                m = q8(ws)
                wi8[i, d, ct] = m
                dw = q8(ws - m)
                wir8[i, d, ct, :, 0:128] = dw[:, 0:128]
                wir8[i, d, ct, :, 128:256] = dw[:, 384:512]
    cwp = np.zeros((2, 8, 128, C), np.float32)
    for ct in range(2):
        for k in range(8):
            cwp[ct, k] = convW[ct * 128:(ct + 1) * 128, :, k]
    bf16 = ml_dtypes.bfloat16
    return (w08.astype(f8), w0r8.astype(f8), wi8.astype(f8), wir8.astype(f8),
            cwp.astype(bf16))


def kernel(**inputs):
    inputs = {k: np.asarray(v) for k, v in inputs.items()}
    x = inputs["x"].astype(np.float32)
    xs = np.ascontiguousarray(
        x.transpose(0, 2, 1, 3).reshape(B * T, C, F_)
    )  # (512, C, F)

    w08, w0r8, wi8, wir8, cwp = _prep_weights(
        inputs["W0"].astype(np.float32),
        [inputs[f"W{i}"].astype(np.float32) for i in (1, 2, 3)],
        inputs["convW"].astype(np.float32),
    )
    bfp = -np.stack([inputs[f"bf{i}"] for i in range(4)]).astype(np.float32)
    brp = np.stack([inputs[f"br{i}"] for i in range(4)]).astype(np.float32)
    gm = inputs["gamma"].reshape(C).astype(np.float32)
    bt = inputs["beta"].reshape(C).astype(np.float32)
    cb = inputs["convb"].reshape(C).astype(np.float32)
    gm2 = np.zeros((2, 128), np.float32)
    gm2[0, 0:64] = gm
    gm2[1, 64:128] = gm
    btc = np.concatenate([bt, bt]).astype(np.float32)

    if "nc" not in _CACHE:
        _CACHE["nc"] = _build()
    nc = _CACHE["nc"]

    shared = {"w08": w08, "w0r8": w0r8, "wi8": wi8, "wir8": wir8, "cwp": cwp,
              "bfp": bfp, "brp": brp, "gm2": gm2, "btc": btc, "cb": cb}
    in_maps = []
    for core in range(NCORES):
        sh = xs[core * NLOC:(core + 1) * NLOC]  # (NLOC, C, F)
        u = np.ascontiguousarray(sh.transpose(1, 0, 2)).reshape(C, NLOC * F_)
        un = np.ascontiguousarray(sh.transpose(0, 2, 1))  # (NLOC, F, C)
        in_maps.append({"u": u, "un": un, **shared})

    trace = bool(os.environ.get("KBENCH_TRACE"))
    res = bass_utils.run_bass_kernel_spmd(
        nc, in_maps, list(range(NCORES)), trace=trace,
        tmpdir=os.environ.get("KBENCH_TMPDIR"),
    )
    _CACHE["last_result"] = res

    full = np.concatenate(
        [res.results[i]["o"].reshape(C, NLOC, F_) for i in range(NCORES)], axis=1
    )  # (C, 512, F)
    out = full.transpose(1, 0, 2).reshape(B, T, C, F_).transpose(0, 2, 1, 3)
    return np.ascontiguousarray(out.astype(np.float32))
